# revision 3
# baseline (speedup 1.0000x reference)
import numpy as np
from contextlib import ExitStack

import concourse.bass as bass
import concourse.tile as tile
from concourse import bacc, mybir
from concourse.bass_utils import run_bass_kernel_spmd

N, C, H, W = 256, 3, 256, 256
D = C * H * W          # 196608
NCORES = 8
RPC = N // NCORES      # 32 rows per core
Q = 4                  # quarters of a row per partition group
P = 128                # partitions = Q * RPC
DPP = D // Q           # 49152 elements per partition
EPS = 1e-6

# Inputs are streamed as fp16 (host downcast) -> DMA floor halves vs fp32.
# Per 8192-chunk engine budget (fp16):
#   DVE: stt z*b (2x) 4.33us + ts zsum (4x) 2.19 + ts bsum (4x) 2.19
#        + stt b*b on (8192-SPLIT) cols (2x) ~2.06          = ~10.8us
#   ACT: Square z (1x) 7.12us + Square b on SPLIT cols ~3.9 = ~11.0us
#   DMA: 2 x 2MB @ ~358GB/s                                 = ~11.7us
CHUNKS = [8192] * 6
assert sum(CHUNKS) == DPP
NCH = len(CHUNKS)
SPLIT = 4352           # b^2 columns on ACT; remainder on DVE

_NC = None


def _build_nc():
    fp32 = mybir.dt.float32
    fp16 = mybir.dt.float16
    AF = mybir.ActivationFunctionType
    ALU = mybir.AluOpType
    AX = mybir.AxisListType

    nc = bacc.Bacc()
    z_ext = nc.dram_tensor("z", [P, DPP], fp16, kind="ExternalInput")
    b_ext = nc.dram_tensor("b", [P, DPP], fp16, kind="ExternalInput")
    out_ext = nc.dram_tensor("out", [P, 6], fp32, kind="ExternalOutput")

    with tile.TileContext(nc) as tc, ExitStack() as ctx:
        zp = ctx.enter_context(tc.tile_pool(name="zp", bufs=3))
        bp = ctx.enter_context(tc.tile_pool(name="bp", bufs=3))
        dp = ctx.enter_context(tc.tile_pool(name="dp", bufs=1))  # DVE scratch
        ap = ctx.enter_context(tc.tile_pool(name="ap", bufs=1))  # ACT scratch
        acc = ctx.enter_context(tc.tile_pool(name="acc", bufs=1))

        CHMAX = max(CHUNKS)
        dscr = dp.tile([P, CHMAX], fp16)
        ascr = ap.tile([P, CHMAX], fp16)

        zb_d = acc.tile([P, NCH], fp32)
        z_d = acc.tile([P, NCH], fp32)
        b_d = acc.tile([P, NCH], fp32)
        bb_d = acc.tile([P, NCH], fp32)
        zz_a = acc.tile([P, NCH], fp32)
        bb_a = acc.tile([P, NCH], fp32)
        stats = acc.tile([P, 6], fp32)

        off = 0
        for i, sz in enumerate(CHUNKS):
            zt = zp.tile([P, sz], fp16)
            nc.sync.dma_start(zt[:], z_ext[:, off:off + sz])
            bt = bp.tile([P, sz], fp16)
            nc.sync.dma_start(bt[:], b_ext[:, off:off + sz])
            off += sz

            sb = min(SPLIT, sz - 2)
            # DVE (z-only op first so it can start before b lands)
            nc.vector.tensor_scalar(
                out=dscr[:, :sz], in0=zt[:], scalar1=1.0, scalar2=None,
                op0=ALU.mult, op1=ALU.add, accum_out=z_d[:, i:i + 1])
            nc.vector.scalar_tensor_tensor(
                out=dscr[:, :sz], in0=zt[:], scalar=1.0, in1=bt[:],
                op0=ALU.mult, op1=ALU.mult, accum_out=zb_d[:, i:i + 1])
            nc.vector.tensor_scalar(
                out=dscr[:, :sz], in0=bt[:], scalar1=1.0, scalar2=None,
                op0=ALU.mult, op1=ALU.add, accum_out=b_d[:, i:i + 1])
            nc.vector.scalar_tensor_tensor(
                out=dscr[:, :sz - sb], in0=bt[:, sb:], scalar=1.0,
                in1=bt[:, sb:], op0=ALU.mult, op1=ALU.mult,
                accum_out=bb_d[:, i:i + 1])
            # ACT
            nc.scalar.activation(out=ascr[:, :sz], in_=zt[:], func=AF.Square,
                                 accum_out=zz_a[:, i:i + 1])
            nc.scalar.activation(out=ascr[:, :sb], in_=bt[:, :sb],
                                 func=AF.Square, accum_out=bb_a[:, i:i + 1])

        # stats cols: [zb, z, b, zz, bb_act, bb_dve]
        nc.vector.tensor_reduce(out=stats[:, 0:1], in_=zb_d[:], axis=AX.X, op=ALU.add)
        nc.vector.tensor_reduce(out=stats[:, 1:2], in_=z_d[:], axis=AX.X, op=ALU.add)
        nc.vector.tensor_reduce(out=stats[:, 2:3], in_=b_d[:], axis=AX.X, op=ALU.add)
        nc.vector.tensor_reduce(out=stats[:, 3:4], in_=zz_a[:], axis=AX.X, op=ALU.add)
        nc.vector.tensor_reduce(out=stats[:, 4:5], in_=bb_a[:], axis=AX.X, op=ALU.add)
        nc.vector.tensor_reduce(out=stats[:, 5:6], in_=bb_d[:], axis=AX.X, op=ALU.add)
        nc.sync.dma_start(out_ext[:], stats[:])

    nc.finalize()
    return nc


def _get_nc():
    global _NC
    if _NC is None:
        _NC = _build_nc()
    return _NC


def _shard(x):
    # [RPC, D] row block -> [P, DPP] where partition p = q*RPC + r owns
    # x[r, q*DPP:(q+1)*DPP]
    return np.ascontiguousarray(
        x.reshape(RPC, Q, DPP).transpose(1, 0, 2).reshape(P, DPP))


def kernel(preds, targets, _trace=False):
    preds = np.ascontiguousarray(preds, dtype=np.float32).reshape(N, D)
    targets = np.ascontiguousarray(targets, dtype=np.float32).reshape(N, D)
    preds16 = preds.astype(np.float16)
    targets16 = targets.astype(np.float16)

    in_maps = []
    for c in range(NCORES):
        rows = slice(c * RPC, (c + 1) * RPC)
        in_maps.append({"z": _shard(targets16[rows]),
                        "b": _shard(preds16[rows])})

    res = run_bass_kernel_spmd(_get_nc(), in_maps, list(range(NCORES)),
                               trace=_trace)
    raw = np.stack([res.results[c]["out"] for c in range(NCORES)])  # [8,P,6]
    raw = raw.astype(np.float64)
    S5 = np.stack([
        raw[..., 1],                 # Sz
        raw[..., 2],                 # Sb
        raw[..., 3],                 # Szz
        raw[..., 4] + raw[..., 5],   # Sbb
        raw[..., 0],                 # Szb
    ], axis=-1)
    S = S5.reshape(NCORES, Q, RPC, 5).sum(axis=1).reshape(N, 5)
    Sz, Sb, Szz, Sbb, Szb = (S[:, j] for j in range(5))
    num = Szb - Sz * Sb / D
    vz = Szz - Sz * Sz / D
    vb = Sbb - Sb * Sb / D
    corr = num / (np.sqrt(vz) * np.sqrt(vb) + EPS)
    out = np.array(corr.mean(), dtype=np.float32)
    if _trace:
        return out, res
    return out


# revision 5
# speedup vs baseline: 1.2922x; 1.2922x over previous
import numpy as np
from contextlib import ExitStack

import concourse.bass as bass
import concourse.tile as tile
from concourse import bacc, mybir
from concourse.bass_utils import run_bass_kernel_spmd

N, C, H, W = 256, 3, 256, 256
D = C * H * W          # 196608
NCORES = 8
RPC = N // NCORES      # 32 rows per core
Q = 4                  # quarters of a row per partition group
P = 128                # partitions = Q * RPC
DPP = D // Q           # 49152 elements per partition
EPS = 1e-6

# Inputs are streamed as fp16 (host downcast) -> DMA floor halves vs fp32.
# Measured engine rates (fp16): DVE stt+accum 0.75 cyc/elem @0.96GHz,
# ACT activation+accum 1 cyc/elem @1.2GHz.  Five reduction passes split:
#   DVE: z*b, sum(z), sum(b) on SPLIT cols   -> 0.75*(2*CH + SPLIT) cyc
#   ACT: z^2, b^2, sum(b) on CH-SPLIT cols   -> 2*CH + (CH-SPLIT) cyc
CHUNKS = [12288] * 4
assert sum(CHUNKS) == DPP
NCH = len(CHUNKS)
SPLIT = 7168           # sum(b) columns on DVE; remainder on ACT

_NC = None


def _build_nc():
    fp32 = mybir.dt.float32
    fp16 = mybir.dt.float16
    AF = mybir.ActivationFunctionType
    ALU = mybir.AluOpType
    AX = mybir.AxisListType

    nc = bacc.Bacc()
    z_ext = nc.dram_tensor("z", [P, DPP], fp16, kind="ExternalInput")
    b_ext = nc.dram_tensor("b", [P, DPP], fp16, kind="ExternalInput")
    out_ext = nc.dram_tensor("out", [P, 6], fp32, kind="ExternalOutput")

    with tile.TileContext(nc) as tc, ExitStack() as ctx:
        zp = ctx.enter_context(tc.tile_pool(name="zp", bufs=3))
        bp = ctx.enter_context(tc.tile_pool(name="bp", bufs=3))
        dp = ctx.enter_context(tc.tile_pool(name="dp", bufs=1))  # DVE scratch
        ap = ctx.enter_context(tc.tile_pool(name="ap", bufs=1))  # ACT scratch
        acc = ctx.enter_context(tc.tile_pool(name="acc", bufs=1))

        CHMAX = max(CHUNKS)
        dscr = dp.tile([P, CHMAX], fp16)
        ascr = ap.tile([P, CHMAX], fp16)

        zb_d = acc.tile([P, NCH], fp32)
        z_d = acc.tile([P, NCH], fp32)
        b_d = acc.tile([P, NCH], fp32)
        bb_d = acc.tile([P, NCH], fp32)
        zz_a = acc.tile([P, NCH], fp32)
        bb_a = acc.tile([P, NCH], fp32)
        stats = acc.tile([P, 6], fp32)

        off = 0
        for i, sz in enumerate(CHUNKS):
            zt = zp.tile([P, sz], fp16)
            nc.sync.dma_start(zt[:], z_ext[:, off:off + sz])
            bt = bp.tile([P, sz], fp16)
            nc.sync.dma_start(bt[:], b_ext[:, off:off + sz])
            off += sz

            sb = min(SPLIT, sz - 2)
            # DVE (z-only ops first so they can start before b lands)
            nc.vector.scalar_tensor_tensor(
                out=dscr[:, :sz], in0=zt[:], scalar=0.0, in1=zt[:],
                op0=ALU.mult, op1=ALU.add, accum_out=z_d[:, i:i + 1])
            nc.vector.scalar_tensor_tensor(
                out=dscr[:, :sz], in0=zt[:], scalar=1.0, in1=bt[:],
                op0=ALU.mult, op1=ALU.mult, accum_out=zb_d[:, i:i + 1])
            nc.vector.scalar_tensor_tensor(
                out=dscr[:, :sb], in0=bt[:, :sb], scalar=0.0, in1=bt[:, :sb],
                op0=ALU.mult, op1=ALU.add, accum_out=bb_d[:, i:i + 1])
            # ACT
            nc.scalar.activation(out=ascr[:, :sz], in_=zt[:], func=AF.Square,
                                 accum_out=zz_a[:, i:i + 1])
            nc.scalar.activation(out=ascr[:, :sz], in_=bt[:], func=AF.Square,
                                 accum_out=bb_a[:, i:i + 1])
            nc.scalar.activation(out=ascr[:, :sz - sb], in_=bt[:, sb:],
                                 func=AF.Copy, accum_out=b_d[:, i:i + 1])

        # stats cols: [zb, z, b, zz, bb_act, bb_dve]
        nc.vector.tensor_reduce(out=stats[:, 0:1], in_=zb_d[:], axis=AX.X, op=ALU.add)
        nc.vector.tensor_reduce(out=stats[:, 1:2], in_=z_d[:], axis=AX.X, op=ALU.add)
        nc.vector.tensor_reduce(out=stats[:, 2:3], in_=b_d[:], axis=AX.X, op=ALU.add)
        nc.vector.tensor_reduce(out=stats[:, 3:4], in_=zz_a[:], axis=AX.X, op=ALU.add)
        nc.vector.tensor_reduce(out=stats[:, 4:5], in_=bb_a[:], axis=AX.X, op=ALU.add)
        nc.vector.tensor_reduce(out=stats[:, 5:6], in_=bb_d[:], axis=AX.X, op=ALU.add)
        nc.sync.dma_start(out_ext[:], stats[:])

    nc.finalize()
    return nc


def _get_nc():
    global _NC
    if _NC is None:
        _NC = _build_nc()
    return _NC


def _shard(x):
    # [RPC, D] row block -> [P, DPP] where partition p = q*RPC + r owns
    # x[r, q*DPP:(q+1)*DPP]
    return np.ascontiguousarray(
        x.reshape(RPC, Q, DPP).transpose(1, 0, 2).reshape(P, DPP))


def kernel(preds, targets, _trace=False):
    preds = np.ascontiguousarray(preds, dtype=np.float32).reshape(N, D)
    targets = np.ascontiguousarray(targets, dtype=np.float32).reshape(N, D)
    preds16 = preds.astype(np.float16)
    targets16 = targets.astype(np.float16)

    in_maps = []
    for c in range(NCORES):
        rows = slice(c * RPC, (c + 1) * RPC)
        in_maps.append({"z": _shard(targets16[rows]),
                        "b": _shard(preds16[rows])})

    res = run_bass_kernel_spmd(_get_nc(), in_maps, list(range(NCORES)),
                               trace=_trace)
    raw = np.stack([res.results[c]["out"] for c in range(NCORES)])  # [8,P,6]
    raw = raw.astype(np.float64)
    S5 = np.stack([
        raw[..., 1],                 # Sz
        raw[..., 2] + raw[..., 5],   # Sb (ACT part + DVE part)
        raw[..., 3],                 # Szz
        raw[..., 4],                 # Sbb
        raw[..., 0],                 # Szb
    ], axis=-1)
    S = S5.reshape(NCORES, Q, RPC, 5).sum(axis=1).reshape(N, 5)
    Sz, Sb, Szz, Sbb, Szb = (S[:, j] for j in range(5))
    num = Szb - Sz * Sb / D
    vz = Szz - Sz * Sz / D
    vb = Sbb - Sb * Sb / D
    corr = num / (np.sqrt(vz) * np.sqrt(vb) + EPS)
    out = np.array(corr.mean(), dtype=np.float32)
    if _trace:
        return out, res
    return out


# revision 8
# speedup vs baseline: 2.0215x; 1.5644x over previous
import numpy as np
from contextlib import ExitStack

import concourse.bass as bass
import concourse.tile as tile
from concourse import bacc, mybir
from concourse.bass_utils import run_bass_kernel_spmd

N, C, H, W = 256, 3, 256, 256
D = C * H * W          # 196608
NCORES = 8
RPC = N // NCORES      # 32 rows per core
Q = 4
P = 128
DPP = D // Q           # 49152 fp16 columns per partition
EPS = 1e-6

# Hybrid layout. Measured fp16 engine rates: DVE tensor_tensor (no accum)
# 0.5 cyc/elem, any accum-bearing DVE op 1.0 cyc/elem, ACT 1.0 cyc/elem
# @1.2GHz, PE matmul 1 moving-col/cyc @2.4GHz. Five reduction passes are
# needed (Sz,Sb,Szz,Sbb,Szb); DVE+ACT alone cannot cover them within the
# DMA window, so part of the data is packed "transposed" (T-layout: d on
# partitions, (block,row) on columns) and reduced on the idle TensorE
# with a ones-stationary matmul accumulating in PSUM.
#   R-segment (X cols):  DVE stt+acc z*b; ACT Sq z, Sq b, Copy z, Copy b
#   T-segment (Y cols):  DVE z*b, z*z, b*b at 2x; PE 5 streams -> PSUM
ROUNDS = 4
CR = 4608              # R cols per round
NSUB = 3
CTS = 2560             # T cols per sub-chunk (5 x 512 matmuls)
CT = NSUB * CTS        # 7680 T cols per round
X = ROUNDS * CR        # 18432
Y = ROUNDS * CT        # 30720
assert X + Y == DPP
D_R = 4 * X            # 73728 elements of each row in R layout
D_T = 4 * Y            # 122880 elements in T layout (960 blocks of 128)
NBLK = D_T // 128      # blocks per row
MMW = 512              # moving cols per matmul
NMM_STREAM = Y // MMW  # 60 matmuls per stream
NSTREAM = 5            # z, b, zb, zz, bb

_NC = None


def _build_nc():
    fp32 = mybir.dt.float32
    fp16 = mybir.dt.float16
    AF = mybir.ActivationFunctionType
    ALU = mybir.AluOpType
    AX = mybir.AxisListType

    nc = bacc.Bacc()
    z_ext = nc.dram_tensor("z", [P, DPP], fp16, kind="ExternalInput")
    b_ext = nc.dram_tensor("b", [P, DPP], fp16, kind="ExternalInput")
    out_ext = nc.dram_tensor("out", [P, 6], fp32, kind="ExternalOutput")
    out2_ext = nc.dram_tensor("out2", [1, NSTREAM * MMW], fp32,
                              kind="ExternalOutput")

    with tile.TileContext(nc) as tc, ExitStack() as ctx:
        zrp = ctx.enter_context(tc.tile_pool(name="zrp", bufs=2))
        brp = ctx.enter_context(tc.tile_pool(name="brp", bufs=2))
        ztp = ctx.enter_context(tc.tile_pool(name="ztp", bufs=2))
        btp = ctx.enter_context(tc.tile_pool(name="btp", bufs=2))
        pp = ctx.enter_context(tc.tile_pool(name="pp", bufs=2))
        dp = ctx.enter_context(tc.tile_pool(name="dp", bufs=1))
        ap = ctx.enter_context(tc.tile_pool(name="ap", bufs=1))
        acc = ctx.enter_context(tc.tile_pool(name="acc", bufs=1))
        ps = ctx.enter_context(tc.psum_pool(name="ps", bufs=1))

        dscr = dp.tile([P, CR], fp16)   # DVE R-scratch
        ascr = ap.tile([P, CR], fp16)   # ACT R-scratch
        ones = acc.tile([P, 1], fp16)
        nc.vector.memset(ones[:], 1.0)

        zbR = acc.tile([P, ROUNDS], fp32)
        zR = acc.tile([P, ROUNDS], fp32)
        bR = acc.tile([P, ROUNDS], fp32)
        zzR = acc.tile([P, ROUNDS], fp32)
        bbR = acc.tile([P, ROUNDS], fp32)
        stats = acc.tile([P, 6], fp32)

        psum = [ps.tile([1, MMW], fp32, name=f"psum{s}")
                for s in range(NSTREAM)]
        mm_idx = [0] * NSTREAM

        def mm(s, src_ap):
            i = mm_idx[s]
            nc.tensor.matmul(psum[s][0:1, :], ones[:, 0:1], src_ap,
                             start=(i == 0), stop=(i == NMM_STREAM - 1))
            mm_idx[s] += 1

        offR = 0
        offT = X
        for rnd in range(ROUNDS):
            zr = zrp.tile([P, CR], fp16)
            nc.sync.dma_start(zr[:], z_ext[:, offR:offR + CR])
            br = brp.tile([P, CR], fp16)
            nc.sync.dma_start(br[:], b_ext[:, offR:offR + CR])
            offR += CR

            # R-segment compute
            nc.vector.scalar_tensor_tensor(
                out=dscr[:], in0=zr[:], scalar=1.0, in1=br[:],
                op0=ALU.mult, op1=ALU.mult, accum_out=zbR[:, rnd:rnd + 1])
            nc.scalar.activation(out=ascr[:], in_=zr[:], func=AF.Square,
                                 accum_out=zzR[:, rnd:rnd + 1])
            nc.scalar.activation(out=ascr[:], in_=zr[:], func=AF.Copy,
                                 accum_out=zR[:, rnd:rnd + 1])
            nc.scalar.activation(out=ascr[:], in_=br[:], func=AF.Square,
                                 accum_out=bbR[:, rnd:rnd + 1])
            nc.scalar.activation(out=ascr[:], in_=br[:], func=AF.Copy,
                                 accum_out=bR[:, rnd:rnd + 1])

            for sub in range(NSUB):
                zt = ztp.tile([P, CTS], fp16)
                nc.sync.dma_start(zt[:], z_ext[:, offT:offT + CTS])
                bt = btp.tile([P, CTS], fp16)
                nc.sync.dma_start(bt[:], b_ext[:, offT:offT + CTS])
                offT += CTS

                # raw-stream matmuls first so PE runs while DVE multiplies
                for blk in range(CTS // MMW):
                    sl = slice(blk * MMW, (blk + 1) * MMW)
                    mm(0, zt[:, sl])
                    mm(1, bt[:, sl])

                pzb = pp.tile([P, CTS], fp16)
                nc.vector.tensor_tensor(out=pzb[:], in0=zt[:], in1=bt[:],
                                        op=ALU.mult)
                pzz = pp.tile([P, CTS], fp16)
                nc.vector.tensor_tensor(out=pzz[:], in0=zt[:], in1=zt[:],
                                        op=ALU.mult)
                pbb = pp.tile([P, CTS], fp16)
                nc.vector.tensor_tensor(out=pbb[:], in0=bt[:], in1=bt[:],
                                        op=ALU.mult)

                for blk in range(CTS // MMW):
                    sl = slice(blk * MMW, (blk + 1) * MMW)
                    mm(2, pzb[:, sl])
                    mm(3, pzz[:, sl])
                    mm(4, pbb[:, sl])

        # R stats cols: [zb, z, b, zz, bb]
        nc.vector.tensor_reduce(out=stats[:, 0:1], in_=zbR[:], axis=AX.X, op=ALU.add)
        nc.vector.tensor_reduce(out=stats[:, 1:2], in_=zR[:], axis=AX.X, op=ALU.add)
        nc.vector.tensor_reduce(out=stats[:, 2:3], in_=bR[:], axis=AX.X, op=ALU.add)
        nc.vector.tensor_reduce(out=stats[:, 3:4], in_=zzR[:], axis=AX.X, op=ALU.add)
        nc.vector.tensor_reduce(out=stats[:, 4:5], in_=bbR[:], axis=AX.X, op=ALU.add)
        nc.vector.tensor_reduce(out=stats[:, 5:6], in_=bbR[:], axis=AX.X, op=ALU.add)
        nc.sync.dma_start(out_ext[:], stats[:])
        tstats = acc.tile([1, NSTREAM * MMW], fp32)
        for s in range(NSTREAM):
            nc.scalar.activation(out=tstats[0:1, s * MMW:(s + 1) * MMW],
                                 in_=psum[s][0:1, :], func=AF.Copy)
        nc.sync.dma_start(out2_ext[:], tstats[:])

    nc.finalize()
    return nc


def _get_nc():
    global _NC
    if _NC is None:
        _NC = _build_nc()
    return _NC


def _pack(x):
    # x: [RPC, D] fp16 row block -> [P, DPP]:
    #  cols [0:X]   R layout: partition q*RPC+r holds quarter q of row r's
    #               first D_R elements
    #  cols [X:DPP] T layout: partition p holds x[r, D_R + k*128 + p] at
    #               column X + k*RPC + r
    rpart = x[:, :D_R].reshape(RPC, Q, X).transpose(1, 0, 2).reshape(P, X)
    tpart = x[:, D_R:].reshape(RPC, NBLK, P).transpose(2, 1, 0).reshape(P, Y)
    return np.ascontiguousarray(np.concatenate([rpart, tpart], axis=1))


def kernel(preds, targets, _trace=False):
    preds = np.ascontiguousarray(preds, dtype=np.float32).reshape(N, D)
    targets = np.ascontiguousarray(targets, dtype=np.float32).reshape(N, D)
    preds16 = preds.astype(np.float16)
    targets16 = targets.astype(np.float16)

    in_maps = []
    for c in range(NCORES):
        rows = slice(c * RPC, (c + 1) * RPC)
        in_maps.append({"z": _pack(targets16[rows]),
                        "b": _pack(preds16[rows])})

    res = run_bass_kernel_spmd(_get_nc(), in_maps, list(range(NCORES)),
                               trace=_trace)
    raw = np.stack([res.results[c]["out"] for c in range(NCORES)])  # [8,P,6]
    raw = raw.astype(np.float64)
    S5_R = np.stack([
        raw[..., 1],   # Sz
        raw[..., 2],   # Sb
        raw[..., 3],   # Szz
        raw[..., 4],   # Sbb
        raw[..., 0],   # Szb
    ], axis=-1)
    S_R = S5_R.reshape(NCORES, Q, RPC, 5).sum(axis=1).reshape(N, 5)

    # T-part: psum[s][c] holds partials for row c % RPC
    raw2 = np.stack([res.results[c]["out2"] for c in range(NCORES)])
    raw2 = raw2.astype(np.float64).reshape(NCORES, NSTREAM, MMW // RPC, RPC)
    ST = raw2.sum(axis=2)  # [NCORES, NSTREAM, RPC]; streams: z,b,zb,zz,bb
    S_T = np.stack([ST[:, 0], ST[:, 1], ST[:, 3], ST[:, 4], ST[:, 2]],
                   axis=-1).reshape(N, 5)

    S = S_R + S_T
    Sz, Sb, Szz, Sbb, Szb = (S[:, j] for j in range(5))
    num = Szb - Sz * Sb / D
    vz = Szz - Sz * Sz / D
    vb = Sbb - Sb * Sb / D
    corr = num / (np.sqrt(vz) * np.sqrt(vb) + EPS)
    out = np.array(corr.mean(), dtype=np.float32)
    if _trace:
        return out, res
    return out


# revision 15
# speedup vs baseline: 2.1491x; 1.0631x over previous
import numpy as np
from contextlib import ExitStack

import concourse.bass as bass
import concourse.tile as tile
from concourse import bacc, mybir
from concourse.bass_utils import run_bass_kernel_spmd

N, C, H, W = 256, 3, 256, 256
D = C * H * W          # 196608
NCORES = 8
RPC = N // NCORES      # 32 rows per core
Q = 4
P = 128
DPP = D // Q           # 49152 fp16 columns per partition
EPS = 1e-6

# Hybrid layout. Measured fp16 engine rates: DVE tensor_tensor (no accum)
# 0.5 cyc/elem, any accum-bearing DVE op 1.0 cyc/elem, ACT 1.0 cyc/elem
# @1.2GHz, PE matmul 1 moving-col/cyc @2.4GHz. Five reduction passes are
# needed (Sz,Sb,Szz,Sbb,Szb); DVE+ACT alone cannot cover them within the
# DMA window, so part of the data is packed "transposed" (T-layout: d on
# partitions, (block,row) on columns) and reduced on the idle TensorE
# with a ones-stationary matmul accumulating in PSUM.
#   R-segment (X cols):  DVE stt+acc z*b; ACT Sq z, Sq b, Copy z, Copy b
#   T-segment (Y cols):  DVE z*b, z*z, b*b at 2x; PE 5 streams -> PSUM
# Interleaved schedule: (kind, cols). Small leading chunks fill the
# pipeline fast; small trailing chunks cut the drain tail.
R_CHUNKS = [1536, 4608, 4608, 4608, 2560]
T_SUBS = [1024] + [2560] * 11 + [1536, 512]
SCHED = [("T", 0), ("R", 0), ("T", 1), ("T", 2), ("R", 1), ("T", 3),
         ("T", 4), ("T", 5), ("R", 2), ("T", 6), ("T", 7), ("T", 8),
         ("R", 3), ("T", 9), ("T", 10), ("T", 11), ("R", 4), ("T", 12),
         ("T", 13)]
X = sum(R_CHUNKS)      # 18432
Y = sum(T_SUBS)        # 30720
CRMAX = max(R_CHUNKS)
assert X + Y == DPP
D_R = 4 * X            # 73728 elements of each row in R layout
D_T = 4 * Y            # 122880 elements in T layout (960 blocks of 128)
NBLK = D_T // 128      # blocks per row
MMW = 512              # moving cols per matmul
NMM_STREAM = Y // MMW  # 60 matmuls per stream
NSTREAM = 5            # z, b, zb, zz, bb

_NC = None


def _build_nc():
    fp32 = mybir.dt.float32
    fp16 = mybir.dt.float16
    AF = mybir.ActivationFunctionType
    ALU = mybir.AluOpType
    AX = mybir.AxisListType

    nc = bacc.Bacc()
    z_ext = nc.dram_tensor("z", [P, DPP], fp16, kind="ExternalInput")
    b_ext = nc.dram_tensor("b", [P, DPP], fp16, kind="ExternalInput")
    out_ext = nc.dram_tensor("out", [P, 6], fp32, kind="ExternalOutput")
    out2_ext = nc.dram_tensor("out2", [1, NSTREAM * MMW], fp32,
                              kind="ExternalOutput")

    with tile.TileContext(nc) as tc, ExitStack() as ctx:
        zrp = ctx.enter_context(tc.tile_pool(name="zrp", bufs=2))
        brp = ctx.enter_context(tc.tile_pool(name="brp", bufs=2))
        ztp = ctx.enter_context(tc.tile_pool(name="ztp", bufs=2))
        btp = ctx.enter_context(tc.tile_pool(name="btp", bufs=2))
        pp = ctx.enter_context(tc.tile_pool(name="pp", bufs=2))
        dp = ctx.enter_context(tc.tile_pool(name="dp", bufs=1))
        ap = ctx.enter_context(tc.tile_pool(name="ap", bufs=1))
        acc = ctx.enter_context(tc.tile_pool(name="acc", bufs=1))
        ps = ctx.enter_context(tc.psum_pool(name="ps", bufs=1))

        dscr = dp.tile([P, CRMAX], fp16)   # DVE R-scratch
        fp8 = mybir.dt.float8e4
        ascr = ap.tile([P, CRMAX], fp8)    # ACT R-scratch (dtype-agnostic rate)
        ones = acc.tile([P, 1], fp16)
        nc.vector.memset(ones[:], 1.0)

        NR = len(R_CHUNKS)
        zbR = acc.tile([P, NR], fp32)
        zR = acc.tile([P, NR], fp32)
        bR = acc.tile([P, NR], fp32)
        zzR = acc.tile([P, NR], fp32)
        bbR = acc.tile([P, NR], fp32)
        stats = acc.tile([P, 6], fp32)

        psum = [ps.tile([1, MMW], fp32, name=f"psum{s}")
                for s in range(NSTREAM)]
        mm_idx = [0] * NSTREAM

        def mm(s, src_ap):
            i = mm_idx[s]
            nc.tensor.matmul(psum[s][0:1, :], ones[:, 0:1], src_ap,
                             start=(i == 0), stop=(i == NMM_STREAM - 1))
            mm_idx[s] += 1

        offR = 0
        offT = X
        for kind, idx in SCHED:
            if kind == "R":
                cr = R_CHUNKS[idx]
                zr = zrp.tile([P, cr], fp16, name="zr")
                nc.sync.dma_start(zr[:], z_ext[:, offR:offR + cr])
                br = brp.tile([P, cr], fp16, name="br")
                nc.sync.dma_start(br[:], b_ext[:, offR:offR + cr])
                offR += cr
                rnd = idx
                nc.vector.scalar_tensor_tensor(
                    out=dscr[:, :cr], in0=zr[:], scalar=1.0, in1=br[:],
                    op0=ALU.mult, op1=ALU.mult,
                    accum_out=zbR[:, rnd:rnd + 1])
                nc.scalar.activation(out=ascr[:, :cr], in_=zr[:],
                                     func=AF.Square,
                                     accum_out=zzR[:, rnd:rnd + 1])
                nc.scalar.activation(out=ascr[:, :cr], in_=zr[:],
                                     func=AF.Copy,
                                     accum_out=zR[:, rnd:rnd + 1])
                nc.scalar.activation(out=ascr[:, :cr], in_=br[:],
                                     func=AF.Square,
                                     accum_out=bbR[:, rnd:rnd + 1])
                nc.scalar.activation(out=ascr[:, :cr], in_=br[:],
                                     func=AF.Copy,
                                     accum_out=bR[:, rnd:rnd + 1])
            else:
                cts = T_SUBS[idx]
                zt = ztp.tile([P, cts], fp16, name="zt")
                nc.sync.dma_start(zt[:], z_ext[:, offT:offT + cts])
                bt = btp.tile([P, cts], fp16, name="bt")
                nc.sync.dma_start(bt[:], b_ext[:, offT:offT + cts])
                offT += cts

                for blk in range(cts // MMW):
                    sl = slice(blk * MMW, (blk + 1) * MMW)
                    mm(0, zt[:, sl])
                    mm(1, bt[:, sl])

                pzb = pp.tile([P, cts], fp16, name="pzb")
                nc.vector.tensor_tensor(out=pzb[:], in0=zt[:], in1=bt[:],
                                        op=ALU.mult)
                pzz = pp.tile([P, cts], fp16, name="pzz")
                nc.vector.tensor_tensor(out=pzz[:], in0=zt[:], in1=zt[:],
                                        op=ALU.mult)
                pbb = pp.tile([P, cts], fp16, name="pbb")
                nc.vector.tensor_tensor(out=pbb[:], in0=bt[:], in1=bt[:],
                                        op=ALU.mult)

                for blk in range(cts // MMW):
                    sl = slice(blk * MMW, (blk + 1) * MMW)
                    mm(2, pzb[:, sl])
                    mm(3, pzz[:, sl])
                    mm(4, pbb[:, sl])

        # R stats cols: [zb, z, b, zz, bb]
        nc.vector.tensor_reduce(out=stats[:, 0:1], in_=zbR[:], axis=AX.X, op=ALU.add)
        nc.vector.tensor_reduce(out=stats[:, 1:2], in_=zR[:], axis=AX.X, op=ALU.add)
        nc.vector.tensor_reduce(out=stats[:, 2:3], in_=bR[:], axis=AX.X, op=ALU.add)
        nc.vector.tensor_reduce(out=stats[:, 3:4], in_=zzR[:], axis=AX.X, op=ALU.add)
        nc.vector.tensor_reduce(out=stats[:, 4:5], in_=bbR[:], axis=AX.X, op=ALU.add)
        nc.sync.dma_start(out_ext[:], stats[:])
        tstats = acc.tile([1, NSTREAM * MMW], fp32)
        for s in range(NSTREAM):
            dst = tstats[0:1, s * MMW:(s + 1) * MMW]
            if s % 2 == 0:
                nc.vector.tensor_copy(dst, psum[s][0:1, :])
            else:
                nc.scalar.activation(out=dst, in_=psum[s][0:1, :],
                                     func=AF.Copy)
        nc.sync.dma_start(out2_ext[:], tstats[:])

    nc.finalize()
    return nc


def _get_nc():
    global _NC
    if _NC is None:
        _NC = _build_nc()
    return _NC


def _pack(x):
    # x: [RPC, D] fp16 row block -> [P, DPP]:
    #  cols [0:X]   R layout: partition q*RPC+r holds quarter q of row r's
    #               first D_R elements
    #  cols [X:DPP] T layout: partition p holds x[r, D_R + k*128 + p] at
    #               column X + k*RPC + r
    rpart = x[:, :D_R].reshape(RPC, Q, X).transpose(1, 0, 2).reshape(P, X)
    tpart = x[:, D_R:].reshape(RPC, NBLK, P).transpose(2, 1, 0).reshape(P, Y)
    return np.ascontiguousarray(np.concatenate([rpart, tpart], axis=1))


def kernel(preds, targets, _trace=False):
    preds = np.ascontiguousarray(preds, dtype=np.float32).reshape(N, D)
    targets = np.ascontiguousarray(targets, dtype=np.float32).reshape(N, D)
    preds16 = preds.astype(np.float16)
    targets16 = targets.astype(np.float16)

    in_maps = []
    for c in range(NCORES):
        rows = slice(c * RPC, (c + 1) * RPC)
        in_maps.append({"z": _pack(targets16[rows]),
                        "b": _pack(preds16[rows])})

    res = run_bass_kernel_spmd(_get_nc(), in_maps, list(range(NCORES)),
                               trace=_trace)
    raw = np.stack([res.results[c]["out"] for c in range(NCORES)])  # [8,P,6]
    raw = raw.astype(np.float64)
    S5_R = np.stack([
        raw[..., 1],   # Sz
        raw[..., 2],   # Sb
        raw[..., 3],   # Szz
        raw[..., 4],   # Sbb
        raw[..., 0],   # Szb
    ], axis=-1)
    S_R = S5_R.reshape(NCORES, Q, RPC, 5).sum(axis=1).reshape(N, 5)

    # T-part: psum[s][c] holds partials for row c % RPC
    raw2 = np.stack([res.results[c]["out2"] for c in range(NCORES)])
    raw2 = raw2.astype(np.float64).reshape(NCORES, NSTREAM, MMW // RPC, RPC)
    ST = raw2.sum(axis=2)  # [NCORES, NSTREAM, RPC]; streams: z,b,zb,zz,bb
    S_T = np.stack([ST[:, 0], ST[:, 1], ST[:, 3], ST[:, 4], ST[:, 2]],
                   axis=-1).reshape(N, 5)

    S = S_R + S_T
    Sz, Sb, Szz, Sbb, Szb = (S[:, j] for j in range(5))
    num = Szb - Sz * Sb / D
    vz = Szz - Sz * Sz / D
    vb = Sbb - Sb * Sb / D
    corr = num / (np.sqrt(vz) * np.sqrt(vb) + EPS)
    out = np.array(corr.mean(), dtype=np.float32)
    if _trace:
        return out, res
    return out
